# revision 1
# baseline (speedup 1.0000x reference)
"""Trainium2 Bass kernel for the CompressibleLoss3 pairwise-logdet loss.

Math: for seg = X[cols] with k rows (k=20 for a pair, k=10 per image),
    logdet(I_256 + c * seg^T seg) = logdet(I_k + c * seg seg^T)
(Weinstein-Aronszajn), so each sampled pair needs only a 20x20 Gram matrix
whose 10x10 diagonal blocks also give the per-image logdets.  Scaling is
folded into host-side constants: logdet(I + c G) = k*ln(c) + logdet(G +
(1/c) I), so the device factors M' = G + eps*I and returns sum(ln pivots).

Distribution: 500 pairs padded to 528 = 8 cores x 66; X replicated (sent
as bf16 unless K_DT=f32 - the Gram/logdet tolerates it).  Per core, per
group of 6 pairs: indirect-gather 120 rows, PE-transpose, then 12 small
matmuls produce the six 20x20 diagonal Gram blocks in a compact [120,20]
layout whose DRAM image makes every readback AP contiguous.  A batched
LDL^T (pairs on partitions) runs split across DVE and Pool with a fused
scalar_tensor_tensor outer product and reciprocal_approx_fast pivots; the
20x20 and both 10x10 factorizations share one reciprocal instruction via
a common 420-element stride layout.  Act only does PSUM->SBUF segT copies
and the final Ln+accum (single act table).

Toolchain note: walrus accepts only ONE embedded semaphore wait per
instruction; _legalize_waits splits extras into standalone EventSemaphore
instructions (skipped under CoreSim).
"""

import math
import os

import numpy as np

# ---- problem constants (hardcoded; kernel.py must be self-contained) ----
NUM_AUG = 10
EPS = 0.01
GAM3 = 0.01
NUM_PAIRS = 500
M_ROWS, N_FEAT = 4000, 256

N_CORES = 8
GROUP_PAIRS = 6            # pairs per matmul group (6*20 = 120 <= 128)
N_GROUPS = 11
B = GROUP_PAIRS * N_GROUPS  # 66 pairs per core (LDL batch, on partitions)
TOTAL_SLOTS = B * N_CORES   # 528 >= 500
K20, K10 = 20, 10
R = GROUP_PAIRS * K20       # 120 gathered rows per group

C20 = float(N_FEAT / ((2.0 * NUM_AUG + 1e-8) * EPS))
C10 = float(N_FEAT / ((1.0 * NUM_AUG + 1e-8) * EPS))
E20 = 1.0 / C20            # diagonal shift for M' = G + (1/c) I
E10 = 1.0 / C10
LNC20 = K20 * math.log(C20)   # host-side logdet constants
LNC10 = K20 * math.log(C10)   # 2 images x 10 pivots

# AB tile layout (per-partition free offsets, f32 elements)
OFF10A = 420               # A10a base; A20 at 0 (20x20, row pitch 20)
OFF10B = 630               # A10b base; S@210, A10a@420, A10b@630: stride 210
ABW = 830

_CACHE = {}


def _build_program(loop_n=1):
    import concourse.bass as bass
    import concourse.mybir as mybir
    import concourse.tile as tile
    from concourse.masks import make_identity

    f32 = mybir.dt.float32
    i32 = mybir.dt.int32
    use_bf16 = os.environ.get("K_DT", "bf16") != "f32"
    dt = mybir.dt.bfloat16 if use_bf16 else f32
    AP = bass.AP
    mult = mybir.AluOpType.mult
    add = mybir.AluOpType.add
    subtract = mybir.AluOpType.subtract
    Ln = mybir.ActivationFunctionType.Ln

    nc = bass.Bass("TRN2", target_bir_lowering=False, debug=False,
                   num_devices=N_CORES)
    X_d = nc.dram_tensor("X", [M_ROWS, N_FEAT], dt, kind="ExternalInput")
    idx_d = nc.dram_tensor("idx", [R, N_GROUPS], i32, kind="ExternalInput")
    out_d = nc.dram_tensor("out", [B, 2], f32, kind="ExternalOutput")

    def flat(t_ap, off, dims):
        """Raw strided AP over a tile's flat [partitions x pitch] space."""
        return AP(t_ap.tensor, t_ap.offset + off, dims)

    def pitch(t_ap):
        return t_ap.ap[0][0]

    with tile.TileContext(nc) as tc:
        with (
            tc.tile_pool(name="const", bufs=1) as constp,
            tc.tile_pool(name="seg", bufs=4) as segp,
            tc.tile_pool(name="segT", bufs=3) as segTp,
            tc.tile_pool(name="ps_t", bufs=3, space="PSUM") as pst,
            tc.tile_pool(name="ps_g", bufs=2, space="PSUM") as psg,
            tc.tile_pool(name="gs", bufs=4) as gsp,
            tc.tile_pool(name="work", bufs=4) as workp,
            tc.tile_pool(name="dstage", bufs=4, space="DRAM") as dstp,
        ):
            ident = constp.tile([128, 128], dt, name="ident")
            make_identity(nc, ident[:, :])
            zeros = constp.tile([B, 1], f32, name="zeros")
            nc.vector.memset(zeros[:, :], 0.0)
            idx_sb = constp.tile([R, N_GROUPS], i32, name="idx_sb")
            nc.sync.dma_start(idx_sb[:, :], idx_d.ap()[:, :])

            # dummy PE consumer of ident: absorbs the gpsimd-compute wait so
            # later transposes carry only their gather-DMA wait
            tp0 = pst.tile([128, 2 * 2 * R], dt, name="tp0", tag="tp")
            nc.tensor.transpose(tp0[:, :128], ident[:, :], ident[:, :])

            # groups are processed as 5 duals + 1 single (11 = 5*2 + 1)
            duals = [(2 * d, 2) for d in range(5)] + [(10, 1)]

            def body():
                AB = workp.tile([B, ABW], f32, name="AB", tag="AB")
                ab, apb = AB[:, :], pitch(AB[:, :])
                dstage = dstp.tile([K20, N_GROUPS * R], f32, name="dstage",
                                   tag="dstage")
                DW = N_GROUPS * R  # 1320, dstage row width

                # one indirect gather for all 11 groups: offsets [120, 11]
                seg = segp.tile([R, N_GROUPS * N_FEAT], dt, name="seg",
                                tag="seg")
                nc.gpsimd.indirect_dma_start(
                    out=seg[:, :], out_offset=None,
                    in_=X_d.ap(),
                    in_offset=bass.IndirectOffsetOnAxis(
                        ap=idx_sb[:, :], axis=0),
                )
                gs = gsp.tile([K20, N_GROUPS * R], f32, name="gs", tag="gs")
                for g0, ng in duals:
                    tp = pst.tile([128, 2 * ng * R], dt, name="tp", tag="tp")
                    for q in range(ng):
                        for h in range(2):
                            nc.tensor.transpose(
                                tp[:, (2 * q + h) * R:(2 * q + h + 1) * R],
                                seg[:, (g0 + q) * N_FEAT + h * 128:
                                       (g0 + q) * N_FEAT + (h + 1) * 128],
                                ident[:R, :R])
                    segT = segTp.tile([128, 2 * ng * R], dt, name="segT",
                                      tag="segT")
                    if (g0 // 2) % 2 == 0:
                        nc.scalar.copy(segT[:, :], tp[:, :])
                    else:
                        nc.vector.tensor_scalar(
                            out=segT[:, :], in0=tp[:, :], scalar1=1.0,
                            scalar2=None, op0=mult)
                    # compact block-diagonal Gram: pair b of group q ->
                    # psum rows 0..19 (PE out base partition must be 0),
                    # cols q*120 + 20b .. +19 within the dual
                    gc = psg.tile([K20, ng * R], f32, name="gc", tag="gc")
                    for q in range(ng):
                        for b in range(GROUP_PAIRS):
                            c0 = 2 * q * R + 20 * b
                            o0 = q * R + K20 * b
                            for h in range(2):
                                nc.tensor.matmul(
                                    gc[0:K20, o0:o0 + K20],
                                    lhsT=segT[:, c0 + h * R:c0 + h * R + 20],
                                    rhs=segT[:, c0 + h * R:c0 + h * R + 20],
                                    start=(h == 0), stop=(h == 1))
                    nc.scalar.copy(gs[:, (g0) * R:(g0 + ng) * R],
                                   gc[:, :])
                # one contiguous write of the whole compact gram staging
                nc.sync.dma_start(dstage[:, :], gs[:, :])

                # --- one merged readback: DRAM dstage -> AB (A20s) ---
                dsab = dstage[:, :]
                # pair p = 6g+b lives at dstage rows 0..19, cols 120g+20b
                src20 = AP(dsab.tensor, dsab.offset,
                           [[R, N_GROUPS], [K20, GROUP_PAIRS],
                            [DW, K20], [1, K20]])
                nc.sync.dma_start(
                    flat(ab, 0, [[apb, B], [K20, K20], [1, K20]]), src20)
                # A10s are the diagonal 10x10 blocks of A20: on-chip copy
                # (Pool - the whole A10 chain lives on Pool, decoupled from
                # the DVE A20 chain)
                nc.gpsimd.tensor_scalar(
                    out=flat(ab, OFF10A,
                             [[apb, B], [210, 2], [K20, K10], [1, K10]]),
                    in0=flat(ab, 0,
                             [[apb, B], [210, 2], [K20, K10], [1, K10]]),
                    scalar1=0.0, scalar2=None, op0=add)

                # --- M' = G + (1/c) I : diagonal shifts only ---
                nc.vector.tensor_scalar(
                    out=flat(ab, 0, [[apb, B], [21, K20]]),
                    in0=flat(ab, 0, [[apb, B], [21, K20]]),
                    scalar1=E20, scalar2=None, op0=add)
                nc.gpsimd.tensor_scalar(
                    out=flat(ab, OFF10A, [[apb, B], [210, 2], [21, K10]]),
                    in0=flat(ab, OFF10A, [[apb, B], [210, 2], [21, K10]]),
                    scalar1=E10, scalar2=None, op0=add)

                # --- batched LDL^T: A20 chain on DVE (divide fused into
                # the outer via stt), A10 chain on Pool - fully decoupled ---
                P = workp.tile([B, (K20 - 1) * (K20 - 1)], f32,
                               name="P", tag="P")
                P2 = workp.tile([B, 3 * (K10 - 1) * (K10 - 1)], f32,
                                name="P2", tag="P2")
                w2 = workp.tile([B, 3 * K10], f32, name="w2", tag="w2")
                apP, aP = pitch(P[:, :]), P[:, :]
                apP2, aP2 = pitch(P2[:, :]), P2[:, :]
                apw, aw = pitch(w2[:, :]), w2[:, :]
                divide = mybir.AluOpType.divide

                # DVE: eliminate cols 0..9 of A20 only.  The trailing
                # 10x10 block is then the Schur complement S whose LDL
                # pivots are exactly A20's remaining pivots, so S joins the
                # two A10s on Pool as three stride-210 matrices.
                invh = workp.tile([B, 1], f32, name="invh", tag="invh")
                for j in range(K10):
                    n = K20 - 1 - j
                    col = (j + 1) * 20 + j
                    trail = flat(ab, (j + 1) * 21, [[apb, B], [20, n], [1, n]])
                    pap = flat(aP, 0, [[apP, B], [n, n], [1, n]])
                    # A20: P = (col_i * (1/d)) * col_k, trail -= P
                    nc.vector.reciprocal(
                        invh[:, 0:1], flat(ab, j * 21, [[apb, B], [1, 1]]))
                    nc.vector.scalar_tensor_tensor(
                        out=pap,
                        in0=flat(ab, col, [[apb, B], [20, n], [0, n]]),
                        scalar=invh[:, 0:1],
                        in1=flat(ab, col, [[apb, B], [0, n], [20, n]]),
                        op0=mult, op1=mult)
                    nc.gpsimd.tensor_tensor(out=trail, in0=trail,
                                             in1=pap, op=subtract)
                # Pool: S (A20 rows/cols 10..19, base 210) + both A10s,
                # all three 10x10 with row pitch 20, base stride 210
                S0 = 210
                invd = workp.tile([B, 3], f32, name="invd", tag="invd")
                for j in range(K10 - 1):
                    m = K10 - 1 - j
                    col1 = S0 + (j + 1) * 20 + j
                    nc.vector.reciprocal(
                        invd[:, 0:3],
                        flat(ab, j * 21 + S0, [[apb, B], [S0, 3]]))
                    nc.gpsimd.tensor_tensor(
                        out=flat(aw, 0, [[apw, B], [K10, 3], [1, m]]),
                        in0=flat(ab, col1,
                                 [[apb, B], [S0, 3], [20, m]]),
                        in1=AP(invd[:, :].tensor, invd[:, :].offset,
                               [[pitch(invd[:, :]), B], [1, 3], [0, m]]),
                        op=mult)
                    nc.gpsimd.tensor_tensor(
                        out=flat(aP2, 0,
                                 [[apP2, B], [m * m, 3], [m, m], [1, m]]),
                        in0=flat(aw, 0,
                                 [[apw, B], [K10, 3], [1, m], [0, m]]),
                        in1=flat(ab, col1,
                                 [[apb, B], [S0, 3], [0, m], [20, m]]),
                        op=mult)
                    trail2 = flat(ab, S0 + (j + 1) * 21,
                                  [[apb, B], [S0, 3], [20, m], [1, m]])
                    nc.gpsimd.tensor_tensor(
                        out=trail2, in0=trail2,
                        in1=flat(aP2, 0,
                                 [[apP2, B], [m * m, 3], [m, m], [1, m]]),
                        op=subtract)

                # --- logdet sums via Ln with accum ---
                lnt = workp.tile([B, K20], f32, name="lnt", tag="lnt")
                osb = workp.tile([B, 2], f32, name="osb", tag="osb")
                nc.scalar.activation(
                    out=lnt[:, :K20],
                    in_=flat(ab, 0, [[apb, B], [21, K20]]),
                    func=Ln, bias=zeros[:, 0:1], accum_out=osb[:, 0:1])
                lf, apL = lnt[:, :], pitch(lnt[:, :])
                nc.scalar.activation(
                    out=flat(lf, 0, [[apL, B], [K10, 2], [1, K10]]),
                    in_=flat(ab, OFF10A, [[apb, B], [210, 2], [21, K10]]),
                    func=Ln, bias=zeros[:, 0:1], accum_out=osb[:, 1:2])
                nc.sync.dma_start(out_d.ap()[:, :], osb[:, 0:2])

            for _ in range(loop_n):
                body()

    if not os.environ.get("K_SIM"):
        _legalize_waits(nc, mybir)
    return nc


def _legalize_waits(nc, mybir):
    """Split multi-wait instructions into standalone single-wait
    EventSemaphore instructions (this toolchain's codegen allows only one
    embedded semaphore wait per instruction)."""
    n_split = 0
    for f in nc.m.functions:
        for blk in f.blocks:
            insts = blk.instructions
            k = 0
            while k < len(insts):
                ins = insts[k]
                si = ins.sync_info
                if si is not None and si.on_wait and len(si.on_wait) > 1:
                    waits = list(si.on_wait)
                    for m, w in enumerate(waits[:-1]):
                        ev = mybir.InstEventSemaphore(
                            name=f"{ins.name}-lw{m}", engine=ins.engine,
                            sync_info=mybir.SyncInfo(on_wait=[w],
                                                     on_update=[]))
                        insts.insert(k, ev)
                        k += 1
                    si.on_wait = [waits[-1]]
                    n_split += 1
                k += 1
    return n_split


def _get_program():
    if "nc" not in _CACHE:
        loop_n = int(os.environ.get("K_LOOP", "1"))
        _CACHE["nc"] = _build_program(loop_n=loop_n)
    return _CACHE["nc"]


def _make_in_maps(X, sample_pairs):
    use_bf16 = os.environ.get("K_DT", "bf16") != "f32"
    if use_bf16:
        import ml_dtypes
        Xc = np.ascontiguousarray(
            np.asarray(X, dtype=np.float32).astype(ml_dtypes.bfloat16))
    else:
        Xc = np.ascontiguousarray(X, dtype=np.float32)
    sp = np.asarray(sample_pairs, dtype=np.int64)
    padded = np.concatenate(
        [sp, np.broadcast_to(sp[:1], (TOTAL_SLOTS - sp.shape[0], 2))], axis=0)
    aug = np.arange(NUM_AUG, dtype=np.int64)
    in_maps = []
    for c in range(N_CORES):
        pc = padded[c * B:(c + 1) * B]                      # [66, 2]
        cols_i = pc[:, 0:1] * NUM_AUG + aug                 # [66, 10]
        cols_j = pc[:, 1:2] * NUM_AUG + aug                 # [66, 10]
        rows = np.concatenate([cols_i, cols_j], axis=1)     # [66, 20]
        # group g holds pairs g*6..g*6+5 -> 120 row indices; idx[p, g]
        idx = rows.reshape(N_GROUPS, R).T
        in_maps.append({
            "X": Xc,
            "idx": np.ascontiguousarray(idx, dtype=np.int32),
        })
    return in_maps


def _postprocess(per_core_outs):
    lds = np.concatenate(per_core_outs, axis=0)[:NUM_PAIRS].astype(np.float64)
    ld_pair = lds[:, 0] + LNC20
    ld_ij = lds[:, 1] + LNC10          # ld_i + ld_j per pair
    ortho = np.mean(ld_pair - 0.5 * ld_ij)
    discrimn = np.mean(ld_pair)
    compress = np.mean(ld_ij)
    total = GAM3 * -ortho
    return np.array([total, discrimn, compress, ortho], dtype=np.float32)


def run_on_hw(X, sample_pairs, trace=False, **spmd_kwargs):
    from concourse.bass_utils import run_bass_kernel_spmd
    nc = _get_program()
    in_maps = _make_in_maps(X, sample_pairs)
    res = run_bass_kernel_spmd(nc, in_maps, core_ids=list(range(N_CORES)),
                               trace=trace, **spmd_kwargs)
    out = _postprocess([r["out"] for r in res.results])
    return out, res


def kernel(X, y=None, sample_pairs=None):
    out, _ = run_on_hw(X, sample_pairs, trace=False)
    return out

